# revision 4
# baseline (speedup 1.0000x reference)
"""Multi-head attention (B=2, S=4096, D=512, H=8) on 8 Trainium2 NeuronCores.

Sharding: batch x head-pair.  Core c handles batch b = c//4 and heads
(2*(c%4), 2*(c%4)+1).  Each core computes its heads' Q/K/V projections,
flash-style attention (scores kept transposed [kv, q] so the attn@V matmul
consumes the exp() output directly, with softmax denominators accumulated via
an extra ones-column on V), and its heads' slice of the output projection.
The 4 per-batch partial outputs are summed on the host (row-parallel linear)
and the output bias is added there.

The kernel is ACT(exp)-bound: softmax exp work is 2*S*S elements per core at
1 elem/lane/cycle.  The schedule therefore streams K/V/Q projections
just-in-time through the first query chunk so exp starts ~10us into the
kernel instead of after all projections, keeps the two heads' score matmuls
packed in the PE array halves (row tiling, K=64), and keeps the output
projection out of the score PSUM banks so qc transitions never stall ACT.
"""

import sys

sys.path.insert(0, "/opt/trn_rl_repo")

import numpy as np
import ml_dtypes

import concourse.bacc as bacc
import concourse.bass as bass
import concourse.tile as tile
from concourse import mybir
from concourse.bass_utils import run_bass_kernel_spmd

BF16 = ml_dtypes.bfloat16

B = 2
S = 4096
D = 512
H = 8
DH = 64           # head dim
HPC = 2           # heads per core
D2 = HPC * DH     # 128, the two heads' feature slice
N_CORES = 8
QC = 512          # query chunk (free dim of scores/attnV matmuls)
KVC = 128         # kv chunk (partition dim of transposed scores)
N_QC = S // QC    # 8
N_KVC = S // KVC  # 32
GROUP = 3         # kv chunks per exp() instruction (PSUM banks per s tile)
KD = D // 128     # 4 contraction chunks of 128

FP32 = mybir.dt.float32
BF16_T = mybir.dt.bfloat16
AF = mybir.ActivationFunctionType


def build_kernel():
    nc = bacc.Bacc("TRN2", debug=False, enable_asserts=False, num_devices=N_CORES)

    qT = nc.dram_tensor("qT", [D, S], BF16_T, kind="ExternalInput").ap()
    kT = nc.dram_tensor("kT", [D, S], BF16_T, kind="ExternalInput").ap()
    vT = nc.dram_tensor("vT", [D, S], BF16_T, kind="ExternalInput").ap()
    wqT2 = nc.dram_tensor("wqT2", [D, D2], BF16_T, kind="ExternalInput").ap()
    wkT2 = nc.dram_tensor("wkT2", [D, D2], BF16_T, kind="ExternalInput").ap()
    wvT2 = nc.dram_tensor("wvT2", [D, D2], BF16_T, kind="ExternalInput").ap()
    wo0 = nc.dram_tensor("wo0", [DH, D], BF16_T, kind="ExternalInput").ap()
    wo1 = nc.dram_tensor("wo1", [DH, D], BF16_T, kind="ExternalInput").ap()
    bq2 = nc.dram_tensor("bq2", [D2, 1], FP32, kind="ExternalInput").ap()
    bk2 = nc.dram_tensor("bk2", [D2, 1], FP32, kind="ExternalInput").ap()
    bv2 = nc.dram_tensor("bv2", [1, D2], BF16_T, kind="ExternalInput").ap()
    ouT = nc.dram_tensor("ouT", [D, S], BF16_T, kind="ExternalOutput").ap()

    with tile.TileContext(nc) as tc:
        with (
            tc.tile_pool(name="persist", bufs=1) as pp,
            tc.tile_pool(name="xq_pool", bufs=3) as pxq,
            tc.tile_pool(name="xk_pool", bufs=4) as pxk,
            tc.tile_pool(name="xv_pool", bufs=2) as pxv,
            tc.tile_pool(name="ptpool", bufs=11) as ppt,
            tc.tile_pool(name="norm", bufs=2) as pn,
            tc.tile_pool(name="outs", bufs=4) as po,
            tc.tile_pool(name="psum", bufs=1, space="PSUM") as psum,
        ):
            # ---- weights / constants to SBUF (first in DMA order) ----
            wq_sb = pp.tile([128, KD, D2], BF16_T)
            wk_sb = pp.tile([128, KD, D2], BF16_T)
            wv_sb = pp.tile([128, KD, D2], BF16_T)
            nc.sync.dma_start(out=wq_sb, in_=wqT2.rearrange("(c p) m -> p c m", p=128))
            nc.sync.dma_start(out=wk_sb, in_=wkT2.rearrange("(c p) m -> p c m", p=128))
            nc.sync.dma_start(out=wv_sb, in_=wvT2.rearrange("(c p) m -> p c m", p=128))
            wo_sb = [pp.tile([DH, D], BF16_T, tag=f"wo{h}", name=f"wo{h}") for h in range(HPC)]
            nc.sync.dma_start(out=wo_sb[0], in_=wo0)
            nc.sync.dma_start(out=wo_sb[1], in_=wo1)
            bq_sb = pp.tile([D2, 1], FP32, tag="bq")
            bk_sb = pp.tile([D2, 1], FP32, tag="bk")
            bv_sb = pp.tile([1, D2], BF16_T, tag="bv")
            nc.sync.dma_start(out=bq_sb, in_=bq2)
            nc.sync.dma_start(out=bk_sb, in_=bk2)
            nc.sync.dma_start(out=bv_sb, in_=bv2)

            # ---- input activations: chunked tiles, critical-first DMA ----
            # xq/xk: [128, 512] per (kc, 512-col chunk); xv: [128, 1024] blocks.
            def xq_tile(c, kc):
                return pxq.tile([128, QC], BF16_T, tag=f"xq{kc}", name=f"xq{kc}_{c}")

            def xk_tile(c, kc):
                return pxk.tile([128, QC], BF16_T, tag=f"xk{kc}", name=f"xk{kc}_{c}")

            def xv_tile(b, kc):
                return pxv.tile([128, 1024], BF16_T, tag=f"xv{kc}", name=f"xv{kc}_{b}")

            xq_tiles = {}
            xk_tiles = {}
            xv_tiles = {}

            def dma_xq(c):
                for kc in range(KD):
                    t = xq_tile(c, kc)
                    nc.sync.dma_start(
                        out=t, in_=qT[kc * 128 : (kc + 1) * 128, c * QC : (c + 1) * QC]
                    )
                    xq_tiles[(c, kc)] = t

            def dma_xk(c):
                for kc in range(KD):
                    t = xk_tile(c, kc)
                    nc.sync.dma_start(
                        out=t, in_=kT[kc * 128 : (kc + 1) * 128, c * QC : (c + 1) * QC]
                    )
                    xk_tiles[(c, kc)] = t

            def dma_xv(b):
                for kc in range(KD):
                    t = xv_tile(b, kc)
                    nc.sync.dma_start(
                        out=t, in_=vT[kc * 128 : (kc + 1) * 128, b * 1024 : (b + 1) * 1024]
                    )
                    xv_tiles[(b, kc)] = t

            # critical-first order: q0, k0, k1 (first score groups), v0, then stream
            dma_xq(0)
            dma_xk(0)
            dma_xk(1)
            dma_xv(0)
            dma_xk(2)
            dma_xk(3)
            dma_xk(4)
            dma_xv(1)

            # ---- constants / misc ----
            bv_bc = pp.tile([128, D2], FP32, tag="bv_bc")
            bv_f32 = pp.tile([1, D2], FP32, tag="bv_f32")
            nc.vector.tensor_copy(out=bv_f32, in_=bv_sb)
            nc.gpsimd.partition_broadcast(bv_bc, bv_f32)
            # warm the ACT exp table before the first real exp
            exp_src = pp.tile([1, 128], FP32, tag="exp_src")
            exp_dst = pp.tile([1, 128], FP32, tag="exp_dst")
            nc.vector.memset(exp_src, 0.0)
            nc.scalar.activation(out=exp_dst, in_=exp_src, func=AF.Exp, scale=1.0)

            # ---- persistent activations ----
            qpT = pp.tile([D2, S], BF16_T, tag="qpT")
            kpT = pp.tile([D2, S], BF16_T, tag="kpT")
            vp = [pp.tile([128, N_KVC, 128], BF16_T, tag=f"vp{h}", name=f"vp{h}") for h in range(HPC)]
            for h in range(HPC):
                nc.vector.memset(vp[h][:, :, DH + 1 :], 0.0)
                nc.vector.memset(vp[h][:, :, DH : DH + 1], 1.0)

            # ---- projection helpers ----
            def proj_qk_chunk(dst, w_sb, b_sb, xin_map, c, tag):
                """Project one 512-col chunk of q or k: dst[:, c*512:(c+1)*512]."""
                pt = psum.tile([D2, QC], FP32, tag=tag, name=f"pt_qk{c}")
                for kc in range(KD):
                    nc.tensor.matmul(
                        pt,
                        w_sb[:, kc, :],
                        xin_map[(c, kc)],
                        start=(kc == 0),
                        stop=(kc == KD - 1),
                    )
                nc.vector.tensor_scalar_add(
                    out=dst[:, c * QC : (c + 1) * QC], in0=pt, scalar1=b_sb
                )

            def proj_v_chunk(sc, tag):
                """Project one 128-col kv chunk of v into vp[h][:, sc, :]."""
                b, off = sc // 8, (sc % 8) * 128
                pt = psum.tile([128, D2], FP32, tag=tag, name=f"pt_v{sc}")
                for kc in range(KD):
                    nc.tensor.matmul(
                        pt,
                        xv_tiles[(b, kc)][:, off : off + 128],
                        wv_sb[:, kc, :],
                        start=(kc == 0),
                        stop=(kc == KD - 1),
                    )
                for h in range(HPC):
                    nc.vector.tensor_add(
                        out=vp[h][:, sc, 0:DH],
                        in0=pt[:, h * DH : (h + 1) * DH],
                        in1=bv_bc[:, h * DH : (h + 1) * DH],
                    )

            # ---- initial projections: qp chunk 0, kp chunk 0 ----
            proj_qk_chunk(qpT, wq_sb, bq_sb, xq_tiles, 0, "s0")
            proj_qk_chunk(kpT, wk_sb, bk_sb, xk_tiles, 0, "s1")

            # ---- stage B: attention + output projection ----
            groups = []
            kv = 0
            while kv < N_KVC:
                n = min(GROUP, N_KVC - kv)
                groups.append((kv, n))
                kv += n
            n_groups = len(groups)

            o_tiles = {}

            def make_attn(qc, g0, glen, p_sbs):
                def emit():
                    for h in range(HPC):
                        if (qc, h) not in o_tiles:
                            o_tiles[(qc, h)] = psum.tile(
                                [128, QC], FP32, tag=f"o{h}", name=f"o_ps{h}"
                            )
                    for gi in range(glen):
                        kvc = g0 + gi
                        for h in range(HPC):
                            nc.tensor.matmul(
                                o_tiles[(qc, h)],
                                vp[h][:, kvc, :],
                                p_sbs[h][:, gi * QC : (gi + 1) * QC],
                                start=(kvc == 0),
                                stop=(kvc == N_KVC - 1),
                            )
                return emit

            def make_norm(qc):
                def emit():
                    ous = []
                    den2 = pn.tile([1, HPC * QC], FP32, tag="den2", name="den2")
                    for h in range(HPC):
                        ou = pn.tile([DH, QC], FP32, tag=f"ou{h}", name=f"ou{h}")
                        nc.vector.tensor_copy(out=ou, in_=o_tiles[(qc, h)][0:DH, :])
                        nc.vector.tensor_copy(
                            out=den2[0:1, h * QC : (h + 1) * QC],
                            in_=o_tiles[(qc, h)][DH : DH + 1, :],
                        )
                        ous.append(ou)
                    rec2 = pn.tile([1, HPC * QC], FP32, tag="rec2", name="rec2")
                    nc.vector.reciprocal_approx_fast(out=rec2, in_=den2)
                    outn = []
                    for h in range(HPC):
                        bcast = pn.tile([DH, QC], FP32, tag=f"bcast{h}", name=f"bcast{h}")
                        nc.gpsimd.partition_broadcast(
                            bcast, rec2[0:1, h * QC : (h + 1) * QC]
                        )
                        on = pn.tile([DH, QC], BF16_T, tag=f"outn{h}", name=f"on{h}")
                        nc.vector.tensor_mul(on, ous[h], bcast)
                        outn.append(on)
                    return outn
                return emit

            def make_proj(qc, outn):
                def emit():
                    qs = slice(qc * QC, (qc + 1) * QC)
                    for ec in range(D // 128):
                        op = psum.tile([128, QC], FP32, tag=f"o{ec % 2}", name="op")
                        nc.tensor.matmul(
                            op, wo_sb[0][:, ec * 128 : (ec + 1) * 128], outn[0],
                            start=True, stop=False,
                        )
                        nc.tensor.matmul(
                            op, wo_sb[1][:, ec * 128 : (ec + 1) * 128], outn[1],
                            start=False, stop=True,
                        )
                        ot = po.tile([128, QC], BF16_T, tag="ot", name="ot")
                        nc.vector.tensor_copy(out=ot, in_=op)
                        nc.sync.dma_start(
                            out=ouT[ec * 128 : (ec + 1) * 128, qs], in_=ot
                        )
                    for h in range(HPC):
                        del o_tiles[(qc, h)]
                return emit

            # software pipeline state
            attn_q = []      # (qc, emit_fn, is_last_group_of_qc)
            LAG = 2
            MAXPOP = 2

            def pump(lag, drain=False):
                pops = 0
                while len(attn_q) > (0 if drain else lag):
                    if not drain and pops >= MAXPOP:
                        break
                    aqc, fn, last = attn_q.pop(0)
                    fn()
                    pops += 1
                    if last:
                        outn = make_norm(aqc)()
                        make_proj(aqc, outn)()

            kp_done = 1      # kp chunks projected so far
            vp_done = 0      # vp kv-chunks projected so far
            xv_dma = 2       # xv blocks DMA'd
            xk_dma = 5       # xk chunks DMA'd
            xq_dma = 1       # xq chunks DMA'd

            def kp_needed(gidx):
                if gidx >= n_groups:
                    return 8
                g0, glen = groups[gidx]
                return (min(N_KVC, g0 + glen) * KVC + QC - 1) // QC

            for qc in range(N_QC):
                qs = slice(qc * QC, (qc + 1) * QC)
                for evi, (g0, glen) in enumerate(groups):
                    # scores: both heads packed into PE array halves
                    s_tiles = [
                        psum.tile([128, glen * QC], FP32, tag=f"s{h}", name=f"s_ps{h}")
                        for h in range(HPC)
                    ]
                    for gi in range(glen):
                        kvc = g0 + gi
                        for h in range(HPC):
                            hs = slice(h * DH, (h + 1) * DH)
                            nc.tensor.matmul(
                                s_tiles[h][:, gi * QC : (gi + 1) * QC],
                                kpT[hs, kvc * KVC : (kvc + 1) * KVC],
                                qpT[hs, qs],
                                start=True,
                                stop=True,
                                tile_position=(h * DH, 0),
                            )
                    p_sbs = []
                    for h in range(HPC):
                        p_sb = ppt.tile(
                            [128, glen * QC], BF16_T, tag=f"pt{h}", name=f"p_sb{h}"
                        )
                        nc.scalar.activation(
                            out=p_sb, in_=s_tiles[h], func=AF.Exp, scale=0.125
                        )
                        p_sbs.append(p_sb)
                    attn_q.append(
                        (qc, make_attn(qc, g0, glen, p_sbs), g0 + glen == N_KVC)
                    )

                    if qc == 0:
                        # JIT kp projection (1-group lookahead) into the s0 slot
                        while kp_done < kp_needed(min(evi + 1, n_groups - 1)):
                            if xk_dma < 8:
                                dma_xk(xk_dma)
                                xk_dma += 1
                            proj_qk_chunk(kpT, wk_sb, bk_sb, xk_tiles, kp_done, "s0")
                            kp_done += 1
                        # JIT vp projection: stay ~2 groups ahead
                        vp_target = min(N_KVC, (evi + 1) * GROUP + 2)
                        while vp_done < vp_target:
                            if vp_done % 8 == 6 and xv_dma < 4:
                                dma_xv(xv_dma)
                                xv_dma += 1
                            proj_v_chunk(vp_done, f"o{vp_done % 2}")
                            vp_done += 1
                    else:
                        pump(LAG)
                    if evi == 9 and qc + 1 < N_QC:
                        # next qc's qp chunk, into the s0 slot
                        while xq_dma <= min(qc + 2, N_QC - 1):
                            dma_xq(xq_dma)
                            xq_dma += 1
                        proj_qk_chunk(qpT, wq_sb, bq_sb, xq_tiles, qc + 1, "s0")
            pump(0, drain=True)
    nc.compile()
    return nc


_NC_CACHE = None


def _get_nc():
    global _NC_CACHE
    if _NC_CACHE is None:
        _NC_CACHE = build_kernel()
    return _NC_CACHE


def make_in_maps(q, k, v, w_q, b_q, w_k, b_k, w_v, b_v, w_o, b_o):
    """Shard the full inputs into the 8 per-core input maps."""
    q = np.asarray(q, np.float32)
    k = np.asarray(k, np.float32)
    v = np.asarray(v, np.float32)
    w_q = np.asarray(w_q, np.float32)
    w_k = np.asarray(w_k, np.float32)
    w_v = np.asarray(w_v, np.float32)
    w_o = np.asarray(w_o, np.float32)
    b_q = np.asarray(b_q, np.float32)
    b_k = np.asarray(b_k, np.float32)
    b_v = np.asarray(b_v, np.float32)

    qT = [np.ascontiguousarray(q[b].T).astype(BF16) for b in range(B)]
    kTb = [np.ascontiguousarray(k[b].T).astype(BF16) for b in range(B)]
    vTb = [np.ascontiguousarray(v[b].T).astype(BF16) for b in range(B)]
    wqT = np.ascontiguousarray(w_q.T).astype(BF16)
    wkT = np.ascontiguousarray(w_k.T).astype(BF16)
    wvT = np.ascontiguousarray(w_v.T).astype(BF16)

    in_maps = []
    for c in range(N_CORES):
        b = c // 4
        hp = c % 4
        js = slice(hp * D2, (hp + 1) * D2)
        h0 = hp * D2
        in_maps.append(
            {
                "qT": qT[b],
                "kT": kTb[b],
                "vT": vTb[b],
                "wqT2": np.ascontiguousarray(wqT[:, js]),
                "wkT2": np.ascontiguousarray(wkT[:, js]),
                "wvT2": np.ascontiguousarray(wvT[:, js]),
                "wo0": np.ascontiguousarray(w_o[:, h0 : h0 + DH].T).astype(BF16),
                "wo1": np.ascontiguousarray(w_o[:, h0 + DH : h0 + 2 * DH].T).astype(BF16),
                "bq2": np.ascontiguousarray(b_q[js].reshape(D2, 1)),
                "bk2": np.ascontiguousarray(b_k[js].reshape(D2, 1)),
                "bv2": np.ascontiguousarray(b_v[js].reshape(1, D2)).astype(BF16),
            }
        )
    return in_maps


def gather_output(results, b_o):
    """Sum per-batch partials, add output bias, restore [B, S, D] layout."""
    b_o = np.asarray(b_o, np.float32)
    out = np.empty((B, S, D), np.float32)
    for b in range(B):
        acc = np.zeros((D, S), np.float32)
        for c in range(b * 4, b * 4 + 4):
            acc += results[c]["ouT"].astype(np.float32)
        out[b] = acc.T + b_o[None, :]
    return out


def kernel(q, k, v, w_q, b_q, w_k, b_k, w_v, b_v, w_o, b_o):
    nc = _get_nc()
    in_maps = make_in_maps(q, k, v, w_q, b_q, w_k, b_k, w_v, b_v, w_o, b_o)
    res = run_bass_kernel_spmd(nc, in_maps, core_ids=list(range(N_CORES)))
    return gather_output(res.results, b_o)


# revision 5
# speedup vs baseline: 1.0104x; 1.0104x over previous
"""Multi-head attention (B=2, S=4096, D=512, H=8) on 8 Trainium2 NeuronCores.

Sharding: batch x head-pair.  Core c handles batch b = c//4 and heads
(2*(c%4), 2*(c%4)+1).  Each core computes its heads' Q/K/V projections,
flash-style attention (scores kept transposed [kv, q] so the attn@V matmul
consumes the exp() output directly, with softmax denominators accumulated via
an extra ones-column on V), and its heads' slice of the output projection.
The 4 per-batch partial outputs are summed on the host (row-parallel linear)
and the output bias is added there.

The kernel is ACT(exp)-bound: softmax exp work is 2*S*S elements per core at
1 elem/lane/cycle.  The schedule therefore streams K/V/Q projections
just-in-time through the first query chunk so exp starts ~10us into the
kernel instead of after all projections, keeps the two heads' score matmuls
packed in the PE array halves (row tiling, K=64), and keeps the output
projection out of the score PSUM banks so qc transitions never stall ACT.
"""

import sys

sys.path.insert(0, "/opt/trn_rl_repo")

import numpy as np
import ml_dtypes

import concourse.bacc as bacc
import concourse.bass as bass
import concourse.tile as tile
from concourse import mybir
from concourse.bass_utils import run_bass_kernel_spmd

BF16 = ml_dtypes.bfloat16

B = 2
S = 4096
D = 512
H = 8
DH = 64           # head dim
HPC = 2           # heads per core
D2 = HPC * DH     # 128, the two heads' feature slice
N_CORES = 8
QC = 512          # query chunk (free dim of scores/attnV matmuls)
KVC = 128         # kv chunk (partition dim of transposed scores)
N_QC = S // QC    # 8
N_KVC = S // KVC  # 32
GROUP = 3         # kv chunks per exp() instruction (PSUM banks per s tile)
KD = D // 128     # 4 contraction chunks of 128

FP32 = mybir.dt.float32
BF16_T = mybir.dt.bfloat16
AF = mybir.ActivationFunctionType


def build_kernel():
    nc = bacc.Bacc("TRN2", debug=False, enable_asserts=False, num_devices=N_CORES)

    qT = nc.dram_tensor("qT", [D, S], BF16_T, kind="ExternalInput").ap()
    kT = nc.dram_tensor("kT", [D, S], BF16_T, kind="ExternalInput").ap()
    vT = nc.dram_tensor("vT", [D, S], BF16_T, kind="ExternalInput").ap()
    wqT2 = nc.dram_tensor("wqT2", [D, D2], BF16_T, kind="ExternalInput").ap()
    wkT2 = nc.dram_tensor("wkT2", [D, D2], BF16_T, kind="ExternalInput").ap()
    wvT2 = nc.dram_tensor("wvT2", [D, D2], BF16_T, kind="ExternalInput").ap()
    wo0 = nc.dram_tensor("wo0", [DH, D], BF16_T, kind="ExternalInput").ap()
    wo1 = nc.dram_tensor("wo1", [DH, D], BF16_T, kind="ExternalInput").ap()
    bq2 = nc.dram_tensor("bq2", [D2, 1], FP32, kind="ExternalInput").ap()
    bk2 = nc.dram_tensor("bk2", [D2, 1], FP32, kind="ExternalInput").ap()
    bv2 = nc.dram_tensor("bv2", [1, D2], BF16_T, kind="ExternalInput").ap()
    ouT = nc.dram_tensor("ouT", [D, S], BF16_T, kind="ExternalOutput").ap()

    with tile.TileContext(nc) as tc:
        with (
            tc.tile_pool(name="persist", bufs=1) as pp,
            tc.tile_pool(name="xq_pool", bufs=3) as pxq,
            tc.tile_pool(name="xk_pool", bufs=4) as pxk,
            tc.tile_pool(name="xv_pool", bufs=2) as pxv,
            tc.tile_pool(name="ptpool", bufs=11) as ppt,
            tc.tile_pool(name="norm", bufs=2) as pn,
            tc.tile_pool(name="outs", bufs=4) as po,
            tc.tile_pool(name="psum", bufs=1, space="PSUM") as psum,
        ):
            # ---- weights / constants to SBUF (first in DMA order) ----
            wq_sb = pp.tile([128, KD, D2], BF16_T)
            wk_sb = pp.tile([128, KD, D2], BF16_T)
            wv_sb = pp.tile([128, KD, D2], BF16_T)
            nc.sync.dma_start(out=wq_sb, in_=wqT2.rearrange("(c p) m -> p c m", p=128))
            nc.sync.dma_start(out=wk_sb, in_=wkT2.rearrange("(c p) m -> p c m", p=128))
            wo_sb = [pp.tile([DH, D], BF16_T, tag=f"wo{h}", name=f"wo{h}") for h in range(HPC)]
            nc.sync.dma_start(out=wo_sb[0], in_=wo0)
            nc.sync.dma_start(out=wo_sb[1], in_=wo1)
            bq_sb = pp.tile([D2, 1], FP32, tag="bq")
            bk_sb = pp.tile([D2, 1], FP32, tag="bk")
            bv_sb = pp.tile([1, D2], BF16_T, tag="bv")
            nc.sync.dma_start(out=bq_sb, in_=bq2)
            nc.sync.dma_start(out=bk_sb, in_=bk2)
            nc.sync.dma_start(out=bv_sb, in_=bv2)

            # ---- input activations: chunked tiles, critical-first DMA ----
            # xq/xk: [128, 512] per (kc, 512-col chunk); xv: [128, 1024] blocks.
            def xq_tile(c, kc):
                return pxq.tile([128, QC], BF16_T, tag=f"xq{kc}", name=f"xq{kc}_{c}")

            def xk_tile(c, kc):
                return pxk.tile([128, QC], BF16_T, tag=f"xk{kc}", name=f"xk{kc}_{c}")

            def xv_tile(b, kc):
                return pxv.tile([128, 1024], BF16_T, tag=f"xv{kc}", name=f"xv{kc}_{b}")

            xq_tiles = {}
            xk_tiles = {}
            xv_tiles = {}

            def dma_xq(c):
                for kc in range(KD):
                    t = xq_tile(c, kc)
                    nc.sync.dma_start(
                        out=t, in_=qT[kc * 128 : (kc + 1) * 128, c * QC : (c + 1) * QC]
                    )
                    xq_tiles[(c, kc)] = t

            def dma_xk(c):
                for kc in range(KD):
                    t = xk_tile(c, kc)
                    nc.sync.dma_start(
                        out=t, in_=kT[kc * 128 : (kc + 1) * 128, c * QC : (c + 1) * QC]
                    )
                    xk_tiles[(c, kc)] = t

            def dma_xv(b):
                for kc in range(KD):
                    t = xv_tile(b, kc)
                    nc.sync.dma_start(
                        out=t, in_=vT[kc * 128 : (kc + 1) * 128, b * 1024 : (b + 1) * 1024]
                    )
                    xv_tiles[(b, kc)] = t

            # critical-first order: k0/q0 (first scores), then v/weights, stream rest
            dma_xk(0)
            dma_xq(0)
            dma_xk(1)
            nc.sync.dma_start(out=wv_sb, in_=wvT2.rearrange("(c p) m -> p c m", p=128))
            dma_xv(0)
            dma_xk(2)
            dma_xk(3)
            dma_xk(4)
            dma_xv(1)

            # ---- constants / misc ----
            bv_bc = pp.tile([128, D2], FP32, tag="bv_bc")
            bv_f32 = pp.tile([1, D2], FP32, tag="bv_f32")
            nc.vector.tensor_copy(out=bv_f32, in_=bv_sb)
            nc.gpsimd.partition_broadcast(bv_bc, bv_f32)
            # warm the ACT exp table before the first real exp
            exp_src = pp.tile([1, 128], FP32, tag="exp_src")
            exp_dst = pp.tile([1, 128], FP32, tag="exp_dst")
            nc.vector.memset(exp_src, 0.0)
            nc.scalar.activation(out=exp_dst, in_=exp_src, func=AF.Exp, scale=1.0)

            # ---- persistent activations ----
            qpT = pp.tile([D2, S], BF16_T, tag="qpT")
            kpT = pp.tile([D2, S], BF16_T, tag="kpT")
            vp = [pp.tile([128, N_KVC, 128], BF16_T, tag=f"vp{h}", name=f"vp{h}") for h in range(HPC)]
            for h in range(HPC):
                nc.vector.memset(vp[h][:, :, DH + 1 :], 0.0)
                nc.vector.memset(vp[h][:, :, DH : DH + 1], 1.0)

            # ---- projection helpers ----
            def proj_qk_chunk(dst, w_sb, b_sb, xin_map, c, tag):
                """Project one 512-col chunk of q or k: dst[:, c*512:(c+1)*512]."""
                pt = psum.tile([D2, QC], FP32, tag=tag, name=f"pt_qk{c}")
                for kc in range(KD):
                    nc.tensor.matmul(
                        pt,
                        w_sb[:, kc, :],
                        xin_map[(c, kc)],
                        start=(kc == 0),
                        stop=(kc == KD - 1),
                    )
                nc.vector.tensor_scalar_add(
                    out=dst[:, c * QC : (c + 1) * QC], in0=pt, scalar1=b_sb
                )

            def proj_v_chunk(sc, tag):
                """Project one 128-col kv chunk of v into vp[h][:, sc, :]."""
                b, off = sc // 8, (sc % 8) * 128
                pt = psum.tile([128, D2], FP32, tag=tag, name=f"pt_v{sc}")
                for kc in range(KD):
                    nc.tensor.matmul(
                        pt,
                        xv_tiles[(b, kc)][:, off : off + 128],
                        wv_sb[:, kc, :],
                        start=(kc == 0),
                        stop=(kc == KD - 1),
                    )
                for h in range(HPC):
                    nc.vector.tensor_add(
                        out=vp[h][:, sc, 0:DH],
                        in0=pt[:, h * DH : (h + 1) * DH],
                        in1=bv_bc[:, h * DH : (h + 1) * DH],
                    )

            # ---- initial projections: qp chunk 0, kp chunk 0 ----
            proj_qk_chunk(qpT, wq_sb, bq_sb, xq_tiles, 0, "s0")
            proj_qk_chunk(kpT, wk_sb, bk_sb, xk_tiles, 0, "s1")

            # ---- stage B: attention + output projection ----
            groups = []
            kv = 0
            while kv < N_KVC:
                n = min(GROUP, N_KVC - kv)
                groups.append((kv, n))
                kv += n
            n_groups = len(groups)

            o_tiles = {}

            def make_attn(qc, g0, glen, p_sbs):
                def emit():
                    for h in range(HPC):
                        if (qc, h) not in o_tiles:
                            o_tiles[(qc, h)] = psum.tile(
                                [128, QC], FP32, tag=f"o{h}", name=f"o_ps{h}"
                            )
                    for gi in range(glen):
                        kvc = g0 + gi
                        for h in range(HPC):
                            nc.tensor.matmul(
                                o_tiles[(qc, h)],
                                vp[h][:, kvc, :],
                                p_sbs[h][:, gi * QC : (gi + 1) * QC],
                                start=(kvc == 0),
                                stop=(kvc == N_KVC - 1),
                            )
                return emit

            def make_norm(qc):
                def emit():
                    ous = []
                    den2 = pn.tile([1, HPC * QC], FP32, tag="den2", name="den2")
                    for h in range(HPC):
                        ou = pn.tile([DH, QC], FP32, tag=f"ou{h}", name=f"ou{h}")
                        nc.vector.tensor_copy(out=ou, in_=o_tiles[(qc, h)][0:DH, :])
                        nc.vector.tensor_copy(
                            out=den2[0:1, h * QC : (h + 1) * QC],
                            in_=o_tiles[(qc, h)][DH : DH + 1, :],
                        )
                        ous.append(ou)
                    rec2 = pn.tile([1, HPC * QC], FP32, tag="rec2", name="rec2")
                    nc.vector.reciprocal_approx_fast(out=rec2, in_=den2)
                    outn = []
                    for h in range(HPC):
                        bcast = pn.tile([DH, QC], FP32, tag=f"bcast{h}", name=f"bcast{h}")
                        nc.gpsimd.partition_broadcast(
                            bcast, rec2[0:1, h * QC : (h + 1) * QC]
                        )
                        on = pn.tile([DH, QC], BF16_T, tag=f"outn{h}", name=f"on{h}")
                        nc.vector.tensor_mul(on, ous[h], bcast)
                        outn.append(on)
                    return outn
                return emit

            def make_proj(qc, outn, ecs):
                def emit():
                    qs = slice(qc * QC, (qc + 1) * QC)
                    for ec in ecs:
                        op = psum.tile([128, QC], FP32, tag=f"o{ec % 2}", name="op")
                        nc.tensor.matmul(
                            op, wo_sb[0][:, ec * 128 : (ec + 1) * 128], outn[0],
                            start=True, stop=False,
                        )
                        nc.tensor.matmul(
                            op, wo_sb[1][:, ec * 128 : (ec + 1) * 128], outn[1],
                            start=False, stop=True,
                        )
                        ot = po.tile([128, QC], BF16_T, tag="ot", name="ot")
                        nc.vector.tensor_copy(out=ot, in_=op)
                        nc.sync.dma_start(
                            out=ouT[ec * 128 : (ec + 1) * 128, qs], in_=ot
                        )
                    if ecs[-1] == D // 128 - 1:
                        for h in range(HPC):
                            del o_tiles[(qc, h)]
                return emit

            # software pipeline state
            attn_q = []      # (qc, emit_fn, is_last_group_of_qc)
            LAG = 2
            MAXPOP = 2
            pending_proj = []   # deferred second half of output projection

            def pump(lag, drain=False):
                if pending_proj:
                    pqc, poutn = pending_proj.pop()
                    make_proj(pqc, poutn, [2, 3])()
                pops = 0
                while len(attn_q) > (0 if drain else lag):
                    if not drain and pops >= MAXPOP:
                        break
                    aqc, fn, last = attn_q.pop(0)
                    fn()
                    pops += 1
                    if last:
                        outn = make_norm(aqc)()
                        make_proj(aqc, outn, [0, 1])()
                        pending_proj.append((aqc, outn))
                if drain and pending_proj:
                    pqc, poutn = pending_proj.pop()
                    make_proj(pqc, poutn, [2, 3])()

            kp_done = 1      # kp chunks projected so far
            vp_done = 0      # vp kv-chunks projected so far
            xv_dma = 2       # xv blocks DMA'd
            xk_dma = 5       # xk chunks DMA'd
            xq_dma = 1       # xq chunks DMA'd

            def kp_needed(gidx):
                if gidx >= n_groups:
                    return 8
                g0, glen = groups[gidx]
                return (min(N_KVC, g0 + glen) * KVC + QC - 1) // QC

            for qc in range(N_QC):
                qs = slice(qc * QC, (qc + 1) * QC)
                for evi, (g0, glen) in enumerate(groups):
                    # scores: both heads packed into PE array halves
                    s_tiles = [
                        psum.tile([128, glen * QC], FP32, tag=f"s{h}", name=f"s_ps{h}")
                        for h in range(HPC)
                    ]
                    for gi in range(glen):
                        kvc = g0 + gi
                        for h in range(HPC):
                            hs = slice(h * DH, (h + 1) * DH)
                            nc.tensor.matmul(
                                s_tiles[h][:, gi * QC : (gi + 1) * QC],
                                kpT[hs, kvc * KVC : (kvc + 1) * KVC],
                                qpT[hs, qs],
                                start=True,
                                stop=True,
                                tile_position=(h * DH, 0),
                            )
                    p_sbs = []
                    for h in range(HPC):
                        p_sb = ppt.tile(
                            [128, glen * QC], BF16_T, tag=f"pt{h}", name=f"p_sb{h}"
                        )
                        nc.scalar.activation(
                            out=p_sb, in_=s_tiles[h], func=AF.Exp, scale=0.125
                        )
                        p_sbs.append(p_sb)
                    attn_q.append(
                        (qc, make_attn(qc, g0, glen, p_sbs), g0 + glen == N_KVC)
                    )

                    if qc == 0:
                        # JIT kp projection (1-group lookahead) into the s0 slot
                        while kp_done < kp_needed(min(evi + 1, n_groups - 1)):
                            if xk_dma < 8:
                                dma_xk(xk_dma)
                                xk_dma += 1
                            proj_qk_chunk(kpT, wk_sb, bk_sb, xk_tiles, kp_done, "s0")
                            kp_done += 1
                        # JIT vp projection: stay ~2 groups ahead
                        vp_target = min(N_KVC, (evi + 1) * GROUP + 2)
                        while vp_done < vp_target:
                            if vp_done % 8 == 6 and xv_dma < 4:
                                dma_xv(xv_dma)
                                xv_dma += 1
                            proj_v_chunk(vp_done, f"o{vp_done % 2}")
                            vp_done += 1
                    else:
                        pump(1 if evi >= 9 else LAG)
                    if evi == 9 and qc + 1 < N_QC:
                        # next qc's qp chunk, into the s0 slot
                        while xq_dma <= min(qc + 2, N_QC - 1):
                            dma_xq(xq_dma)
                            xq_dma += 1
                        proj_qk_chunk(qpT, wq_sb, bq_sb, xq_tiles, qc + 1, "s0")
            pump(0, drain=True)
    nc.compile()
    return nc


_NC_CACHE = None


def _get_nc():
    global _NC_CACHE
    if _NC_CACHE is None:
        _NC_CACHE = build_kernel()
    return _NC_CACHE


def make_in_maps(q, k, v, w_q, b_q, w_k, b_k, w_v, b_v, w_o, b_o):
    """Shard the full inputs into the 8 per-core input maps."""
    q = np.asarray(q, np.float32)
    k = np.asarray(k, np.float32)
    v = np.asarray(v, np.float32)
    w_q = np.asarray(w_q, np.float32)
    w_k = np.asarray(w_k, np.float32)
    w_v = np.asarray(w_v, np.float32)
    w_o = np.asarray(w_o, np.float32)
    b_q = np.asarray(b_q, np.float32)
    b_k = np.asarray(b_k, np.float32)
    b_v = np.asarray(b_v, np.float32)

    qT = [np.ascontiguousarray(q[b].T).astype(BF16) for b in range(B)]
    kTb = [np.ascontiguousarray(k[b].T).astype(BF16) for b in range(B)]
    vTb = [np.ascontiguousarray(v[b].T).astype(BF16) for b in range(B)]
    wqT = np.ascontiguousarray(w_q.T).astype(BF16)
    wkT = np.ascontiguousarray(w_k.T).astype(BF16)
    wvT = np.ascontiguousarray(w_v.T).astype(BF16)

    in_maps = []
    for c in range(N_CORES):
        b = c // 4
        hp = c % 4
        js = slice(hp * D2, (hp + 1) * D2)
        h0 = hp * D2
        in_maps.append(
            {
                "qT": qT[b],
                "kT": kTb[b],
                "vT": vTb[b],
                "wqT2": np.ascontiguousarray(wqT[:, js]),
                "wkT2": np.ascontiguousarray(wkT[:, js]),
                "wvT2": np.ascontiguousarray(wvT[:, js]),
                "wo0": np.ascontiguousarray(w_o[:, h0 : h0 + DH].T).astype(BF16),
                "wo1": np.ascontiguousarray(w_o[:, h0 + DH : h0 + 2 * DH].T).astype(BF16),
                "bq2": np.ascontiguousarray(b_q[js].reshape(D2, 1)),
                "bk2": np.ascontiguousarray(b_k[js].reshape(D2, 1)),
                "bv2": np.ascontiguousarray(b_v[js].reshape(1, D2)).astype(BF16),
            }
        )
    return in_maps


def gather_output(results, b_o):
    """Sum per-batch partials, add output bias, restore [B, S, D] layout."""
    b_o = np.asarray(b_o, np.float32)
    out = np.empty((B, S, D), np.float32)
    for b in range(B):
        acc = np.zeros((D, S), np.float32)
        for c in range(b * 4, b * 4 + 4):
            acc += results[c]["ouT"].astype(np.float32)
        out[b] = acc.T + b_o[None, :]
    return out


def kernel(q, k, v, w_q, b_q, w_k, b_k, w_v, b_v, w_o, b_o):
    nc = _get_nc()
    in_maps = make_in_maps(q, k, v, w_q, b_q, w_k, b_k, w_v, b_v, w_o, b_o)
    res = run_bass_kernel_spmd(nc, in_maps, core_ids=list(range(N_CORES)))
    return gather_output(res.results, b_o)
